# revision 7
# baseline (speedup 1.0000x reference)
"""Trainium2 Bass kernel for nn_CompoundDecoder: 3-layer LSTM greedy decoder.

Strategy: pure data-parallel over batch (2048 -> 8 cores x 256). All weights
resident in SBUF (feature-major layout). 99 decode steps fully unrolled.
fp32 matmuls on PE; gates via ACT Sigmoid/Tanh splines; cell pointwise on DVE;
embedding gather via one-hot matmul against a host-precomputed table
E0T = emb @ w_ih0[:, :256].T + (b_ih0 + b_hh0); argmax via PE transpose +
DVE max/max_index.

Walrus constraint: at most ONE sync wait per instruction. Structure keeps each
instruction's new-proc-tick count <= 1; targeted engine nops absorb joins.
"""
import numpy as np

import bass_rust
import concourse.bass as bass
import concourse.tile as tile
from concourse import mybir
from concourse.bass_utils import run_bass_kernel_spmd

f32 = mybir.dt.float32
i32 = mybir.dt.int32
u32 = mybir.dt.uint32
AF = mybir.ActivationFunctionType
OP = mybir.AluOpType

VOCAB = 100
HIDDEN = 256
IN_DIM = 512
LAYERS = 3
MAX_LEN = 100
BATCH = 2048
NCORES = 8
BL = BATCH // NCORES          # 256 batch rows per core
NSTEPS = MAX_LEN - 1          # 99 decode steps


def _pack_rows(a):
    """[K, C] (K multiple of 128) -> [128, (K//128)*C]; k-tile j at cols j*C:(j+1)*C."""
    K, C = a.shape
    assert K % 128 == 0
    return np.ascontiguousarray(
        a.reshape(K // 128, 128, C).transpose(1, 0, 2).reshape(128, -1)).astype(np.float32)


def _pack_vec(v):
    """[F] (F multiple of 128) -> [128, F//128]; col m holds features m*128:(m+1)*128."""
    F = v.shape[0]
    return np.ascontiguousarray(v.reshape(F // 128, 128).T).astype(np.float32)


class _Layout:
    """Column-range allocator for the const blob."""
    def __init__(self):
        self.cols = 0
        self.ranges = {}

    def add(self, name, arr128):
        assert arr128.shape[0] == 128
        c = arr128.shape[1]
        self.ranges[name] = (self.cols, c)
        self.cols += c
        return arr128


def _build_blob(emb, w_lat, b_lat, w_out, b_out, lstm_params):
    L = _Layout()
    parts = []

    w_ih0, w_hh0, b_ih0, b_hh0 = lstm_params[0]
    b0 = (b_ih0 + b_hh0).astype(np.float32)

    # E0T' [128, 1024]: rows v=0..99 -> emb[v] @ w_ih0[:, :256].T + b0
    e0 = emb.astype(np.float64) @ w_ih0[:, :HIDDEN].astype(np.float64).T
    e0 = e0.astype(np.float32) + b0[None, :]
    e0p = np.zeros((128, 4 * HIDDEN), np.float32)
    e0p[:VOCAB] = e0
    parts.append(L.add("E0T", e0p))

    # w_z: w_ih0[:, 256:768].T [512, 1024] packed
    parts.append(L.add("WZ", _pack_rows(w_ih0[:, HIDDEN:].T.copy())))
    # w_hh l and w_ih l (l=1,2): [256, 1024] transposed packed
    for l in range(LAYERS):
        wi, wh, bi, bh = lstm_params[l]
        parts.append(L.add(f"WHH{l}", _pack_rows(wh.T.copy())))
        if l > 0:
            parts.append(L.add(f"WIH{l}", _pack_rows(wi.T.copy())))
            parts.append(L.add(f"B{l}", _pack_vec((bi + bh).astype(np.float32))))
    parts.append(L.add("WLAT", _pack_rows(w_lat.T.copy())))
    parts.append(L.add("BLAT", _pack_vec(b_lat.astype(np.float32))))
    wop = np.zeros((128, 2 * VOCAB), np.float32)
    wot = _pack_rows(w_out.T.copy())          # [128, 2*100]
    wop[:, :] = wot
    parts.append(L.add("WOUT", wop))
    bo = np.zeros((128, 1), np.float32)
    bo[:VOCAB, 0] = b_out
    parts.append(L.add("BOUT", bo))
    parts.append(L.add("I128", np.eye(128, dtype=np.float32)))
    parts.append(L.add("IOTA", np.tile(np.arange(VOCAB, dtype=np.float32), (128, 1))))
    oh1 = np.zeros((128, BL), np.float32)
    oh1[1, :] = 1.0                            # bos token = 1
    parts.append(L.add("OH0", oh1))
    parts.append(L.add("EOSC", np.ones((128, 2), np.float32)))
    parts.append(L.add("PADS", np.full((128, 2), float(MAX_LEN), np.float32)))
    parts.append(L.add("CNT", np.full((128, 1), 2.0, np.float32)))
    tok = np.zeros((128, 2 * (MAX_LEN + 1)), np.float32)
    tok[:, 0] = 1.0
    tok[:, MAX_LEN + 1] = 1.0                  # bos column for each half
    parts.append(L.add("TOK", tok))

    blob = np.concatenate(parts, axis=1)
    return blob, L


def _dep(a, b):
    bass_rust.add_dep_helper(a.ins, b.ins, sync=True, reason="join")


def _build_bass(blob, ranges, nsteps=NSTEPS):
    const_cols = blob.shape[1]
    nc = bass.Bass(trn_type="TRN2", target_bir_lowering=False)
    consts_d = nc.inline_tensor(blob, name="consts")
    z_d = nc.dram_tensor("zt_in", [128, 4 * BL], f32, kind="ExternalInput")
    tok_d = nc.dram_tensor("out_tokens", [BL, MAX_LEN], i32, kind="ExternalOutput")
    pads_d = nc.dram_tensor("out_pads", [BL], i32, kind="ExternalOutput")

    tc_obj = tile.TileContext(nc)
    with tc_obj as tc:
        _build_body(nc, tc, consts_d, z_d, tok_d, pads_d, const_cols, ranges, nsteps)
    return nc


def _build_body(nc, tc, consts_d, z_d, tok_d, pads_d, const_cols, ranges, nsteps):
    import contextlib
    ctx = contextlib.ExitStack()
    with ctx:
        sb = ctx.enter_context(tc.tile_pool(name="sb", bufs=1))
        ps = ctx.enter_context(tc.tile_pool(name="ps", bufs=1, space="PSUM"))

        cblob = sb.tile([128, const_cols], f32, name="cblob")
        dma_c = nc.sync.dma_start(cblob[:], consts_d.ap()[:, :])

        def R(name, rows=128):
            s, n = ranges[name]
            return cblob[0:rows, s:s + n]

        def Rcol(name, c0, n, rows=128):
            s, _ = ranges[name]
            return cblob[0:rows, s + c0:s + c0 + n]


        # --- PSUM tensors ---
        g = [ps.tile([128, 2 * BL], f32, name=f"g{i}") for i in range(4)]
        lg_ps = ps.tile([128, BL], f32, name="lg_ps")
        tr_ps = ps.tile([128, BL], f32, name="tr_ps")
        oh_ps = ps.tile([128, BL], f32, name="oh_ps")
        h0_ps = ps.tile([128, 2 * BL], f32, name="h0_ps")

        # --- persistent SBUF tiles ---
        zc = [sb.tile([128, 2 * BL], f32, name=f"zc{i}") for i in range(4)]
        # states: double-buffered by step parity
        H = [[sb.tile([128, 2 * BL], f32, name=f"H{l}_{p}") for p in range(2)]
             for l in range(LAYERS)]
        C = [[sb.tile([128, 2 * BL], f32, name=f"C{l}_{p}") for p in range(2)]
             for l in range(LAYERS)]
        pre0 = [sb.tile([128, 2 * BL], f32, name=f"pre0_{i}") for i in range(4)]
        # gate output tiles, double-buffered by layer parity
        gt = [[sb.tile([128, 2 * BL], f32, name=f"gt{i}_{p}") for p in range(2)]
              for i in range(4)]
        tcn = [sb.tile([128, 2 * BL], f32, name=f"tc{p}") for p in range(2)]
        m1 = [sb.tile([128, 2 * BL], f32, name=f"m1_{p}") for p in range(2)]
        m2 = [sb.tile([128, 2 * BL], f32, name=f"m2_{p}") for p in range(2)]
        logits_bv = sb.tile([128, 2 * 128], f32, name="logits_bv")
        logits_tr = sb.tile([128, 2 * VOCAB], f32, name="logits_tr")
        onehot = sb.tile([128, BL], f32, name="onehot")
        oh_bv = sb.tile([128, 2 * VOCAB], f32, name="oh_bv")
        mx8 = sb.tile([128, 16], f32, name="mx8")
        idx8 = sb.tile([128, 16], u32, name="idx8")
        w_f = sb.tile([128, 2], f32, name="w_f")
        is2 = sb.tile([128, 2], f32, name="is2")
        ieos = sb.tile([128, 2], f32, name="ieos")
        dtmp = sb.tile([128, 2], f32, name="dtmp")
        tok_i = sb.tile([128, 2 * (MAX_LEN + 1)], i32, name="tok_i")
        pads_i = sb.tile([128, 2], i32, name="pads_i")

        eosc = R("EOSC")
        pads = R("PADS")
        cnt = R("CNT")
        toksb = R("TOK")

        # ============ PRECOMPUTE ============
        ztt = sb.tile([128, 4 * BL], f32, name="ztt")
        dma_z = nc.sync.dma_start(ztt[:], z_d.ap()[:, :])
        zt = ztt[:]
        scr = sb.tile([128, 8], f32, name="scr")
        # ACT observes the consts DMA via a tiny real op
        nc.scalar.copy(scr[0:1, 0:1], cblob[0:1, 0:1])
        # PE observes the z DMA via a tiny real matmul (nop lives on seq proc)
        nc.tensor.matmul(lg_ps[0:1, 0:1], ztt[0:1, 0:1], ztt[0:1, 0:1],
                         start=True, stop=True)

        # h0 = w_lat @ z^T  -> h0_ps [128, 512]  (M-tile m at cols m*BL)
        for m in range(2):
            for k in range(4):
                nc.tensor.matmul(
                    h0_ps[:, m * BL:(m + 1) * BL],
                    Rcol("WLAT", k * HIDDEN + m * 128, 128),
                    zt[:, k * BL:(k + 1) * BL],
                    start=(k == 0), stop=(k == 3))
        # zc[gi] = w_z @ z^T  (M-tile mm = gi*2+m at cols m*BL of zc[gi])
        for gi in range(4):
            for m in range(2):
                mm = gi * 2 + m
                for k in range(4):
                    nc.tensor.matmul(
                        g[gi][:, m * BL:(m + 1) * BL],
                        Rcol("WZ", k * 1024 + mm * 128, 128),
                        zt[:, k * BL:(k + 1) * BL],
                        start=(k == 0), stop=(k == 3))
        # ACT: states init H/C = h0 + b_lat; zc copies
        for l in range(LAYERS):
            for m in range(2):
                nc.scalar.activation(H[l][0][:, m * BL:(m + 1) * BL],
                                     h0_ps[:, m * BL:(m + 1) * BL],
                                     AF.Identity, bias=Rcol("BLAT", m, 1))
                nc.scalar.activation(C[l][0][:, m * BL:(m + 1) * BL],
                                     h0_ps[:, m * BL:(m + 1) * BL],
                                     AF.Identity, bias=Rcol("BLAT", m, 1))
        last_zc = None
        for gi in range(4):
            last_zc = nc.scalar.copy(zc[gi][:], g[gi][:])

        # DVE observes the consts DMA and the ACT precompute tail
        nc.vector.tensor_copy(scr[0:1, 1:2], cblob[0:1, 0:1])
        nc.vector.tensor_copy(scr[0:1, 2:3], zc[3][0:1, 0:1])

        onehot_cur = R("OH0", rows=VOCAB)

        # ============ DECODE STEPS ============
        for t in range(1, nsteps + 1):
            pp = t % 2          # state buffer parity: step t writes H[l][pp]
            qq = 1 - pp         # reads H[l][qq]
            Hp = [H[l][qq] for l in range(LAYERS)]
            Cp = [C[l][qq] for l in range(LAYERS)]
            Hn = [H[l][pp] for l in range(LAYERS)]
            Cn = [C[l][pp] for l in range(LAYERS)]
            gp = t % 2          # gate tile parity per layer below

            for l in range(LAYERS):
                glp = (t * LAYERS + l) % 2
                # ---- PE: gates ----
                if l == 0:
                    for gi in range(4):
                        for m in range(2):
                            mm = gi * 2 + m
                            dst = g[gi][:, m * BL:(m + 1) * BL]
                            nc.tensor.matmul(
                                dst, Rcol("E0T", mm * 128, 128, rows=VOCAB),
                                onehot_cur, start=True, stop=False)
                            for k in range(2):
                                nc.tensor.matmul(
                                    dst, Rcol("WHH0", k * 1024 + mm * 128, 128),
                                    Hp[0][:, k * BL:(k + 1) * BL],
                                    start=False, stop=(k == 1))
                else:
                    if l == 2:
                        # absorb ACT tick (psum WAR vs L1 sigma reads) with a
                        # tiny real matmul (nops live on a separate seq proc)
                        glp1 = (t * LAYERS + 1) % 2
                        nc.tensor.matmul(lg_ps[0:1, 0:1], gt[3][glp1][0:1, 0:1],
                                         gt[3][glp1][0:1, 0:1], start=True, stop=True)
                    xin = Hn[l - 1]
                    for gi in range(4):
                        for m in range(2):
                            mm = gi * 2 + m
                            dst = g[gi][:, m * BL:(m + 1) * BL]
                            for k in range(2):
                                nc.tensor.matmul(
                                    dst, Rcol(f"WIH{l}", k * 1024 + mm * 128, 128),
                                    xin[:, k * BL:(k + 1) * BL],
                                    start=(k == 0), stop=False)
                            for k in range(2):
                                nc.tensor.matmul(
                                    dst, Rcol(f"WHH{l}", k * 1024 + mm * 128, 128),
                                    Hp[l][:, k * BL:(k + 1) * BL],
                                    start=False, stop=(k == 1))

                # ---- pre-activations + nonlinearities ----
                # gate order in rows: i, f, g, o -> gi 0..3
                sig_t = [None] * 4
                if l == 0:
                    for gi in range(4):
                        nc.vector.scalar_tensor_tensor(
                            pre0[gi][:], g[gi][:], 1.0, zc[gi][:], OP.mult, OP.add)
                    for gi, fn in ((0, AF.Sigmoid), (1, AF.Sigmoid), (2, AF.Tanh), (3, AF.Sigmoid)):
                        sig_t[gi] = nc.scalar.activation(gt[gi][glp][:], pre0[gi][:], fn)
                else:
                    for gi, fn in ((0, AF.Sigmoid), (1, AF.Sigmoid), (2, AF.Tanh), (3, AF.Sigmoid)):
                        for m in range(2):
                            mm = gi * 2 + m
                            sig_t[gi] = nc.scalar.activation(
                                gt[gi][glp][:, m * BL:(m + 1) * BL],
                                g[gi][:, m * BL:(m + 1) * BL],
                                fn, bias=Rcol(f"B{l}", mm, 1))
                last_act_sig = sig_t[3]

                # ---- DVE cell update ----
                i_m2 = nc.vector.tensor_mul(m2[glp][:], gt[0][glp][:], gt[2][glp][:])
                i_m1 = nc.vector.tensor_mul(m1[glp][:], gt[1][glp][:], Cp[l][:])
                bass_rust.add_dep_helper(i_m1.ins, i_m2.ins, sync=False,
                                         reason="order m1 after m2")
                nc.vector.tensor_add(Cn[l][:], m1[glp][:], m2[glp][:])
                nc.scalar.activation(tcn[glp][:], Cn[l][:], AF.Tanh)
                nc.vector.tensor_mul(Hn[l][:], gt[3][glp][:], tcn[glp][:])

            # ---- logits ----
            for k in range(2):
                nc.tensor.matmul(
                    lg_ps[0:VOCAB, :], Rcol("WOUT", k * VOCAB, VOCAB),
                    Hn[2][:, k * BL:(k + 1) * BL],
                    start=(k == 0), stop=(k == 1))
            act_lg = nc.scalar.activation(logits_bv[0:VOCAB, 0:BL], lg_ps[0:VOCAB, :],
                                          AF.Identity, bias=R("BOUT", rows=VOCAB))
            # transpose halves: [100, 128] -> [128, 100]
            for h in range(2):
                nc.tensor.matmul(
                    tr_ps[:, h * 100:(h + 1) * 100][0:128, :],
                    logits_bv[0:VOCAB, h * 128:(h + 1) * 128],
                    Rcol("I128", 0, 100, rows=VOCAB),
                    is_transpose=True)
            act_tr = nc.scalar.copy(logits_tr[:, 0:200], tr_ps[:, 0:200])

            # ---- argmax ----
            for h in range(2):
                nc.vector.max(mx8[:, h * 8:(h + 1) * 8], logits_tr[:, h * 100:h * 100 + 100])
                nc.vector.max_index(idx8[:, h * 8:(h + 1) * 8], mx8[:, h * 8:(h + 1) * 8],
                                    logits_tr[:, h * 100:h * 100 + 100])
            for h in range(2):
                nc.vector.tensor_copy(w_f[:, h:h + 1], idx8[:, h * 8:h * 8 + 1])

            # ---- one-hot for next step ----
            if t < nsteps:
                for h in range(2):
                    nc.vector.tensor_scalar(oh_bv[:, h * VOCAB:(h + 1) * VOCAB],
                                            R("IOTA"), w_f[:, h:h + 1], None, OP.is_equal)
                for h in range(2):
                    nc.tensor.matmul(
                        oh_ps[0:VOCAB, h * 128:(h + 1) * 128],
                        oh_bv[:, h * VOCAB:(h + 1) * VOCAB],
                        R("I128"),
                        is_transpose=True)
                act_oh = nc.scalar.copy(onehot[0:VOCAB, :], oh_ps[0:VOCAB, :])
                onehot_cur = onehot[0:VOCAB, :]

            # ---- token bookkeeping (DVE) ----
            nc.vector.tensor_scalar(is2[:], w_f[:], 2.0, None, OP.is_equal)
            for h in range(2):
                nc.vector.tensor_mul(toksb[:, h * (MAX_LEN + 1) + t:h * (MAX_LEN + 1) + t + 1],
                                     w_f[:, h:h + 1], eosc[:, h:h + 1])
            nc.vector.tensor_mul(ieos[:], is2[:], eosc[:])
            nc.vector.tensor_scalar(dtmp[:], pads[:], cnt[:, 0:1], None, OP.subtract)
            nc.vector.tensor_mul(dtmp[:], ieos[:], dtmp[:])
            nc.vector.tensor_sub(pads[:], pads[:], dtmp[:])
            nc.vector.tensor_sub(eosc[:], eosc[:], ieos[:])
            nc.vector.tensor_scalar(cnt[:], cnt[:], 1.0, None, OP.add)

        # ============ OUTPUT ============
        nc.vector.tensor_copy(tok_i[:], toksb[:])
        nc.vector.tensor_copy(pads_i[:], pads[:])
        # tokens: out[h*128+p, j] = tok_i[p, h*(MAX_LEN+1) + j], j in 0..99
        dma_o1 = nc.sync.dma_start(
            tok_d.ap().rearrange("(h p) j -> p h j", p=128),
            tok_i[:].rearrange("p (h j) -> p h j", h=2)[:, :, 0:MAX_LEN])
        dma_o2 = nc.sync.dma_start(
            pads_d.ap().rearrange("(h p) -> p h", p=128), pads_i[:])

        # ============ TAIL ABSORB ============
        _absorb_all(nc)


def _absorb_all(nc):
    insts = list(nc.cur_bb.bb.instructions)
    by_engine = {}
    dmas = []
    for ins in insts:
        nm = type(ins).__name__
        if not ins.is_executable():
            continue
        if nm in ("InstDMACopy", "InstDMATranspose", "InstTriggerDma"):
            dmas.append(ins)
        else:
            by_engine.setdefault(ins.engine, []).append(ins)
    for d in dmas:
        nop = nc.sync.nop()
        bass_rust.add_dep_helper(nop.ins, d, sync=True, reason="absorb dma")
    for eng, group in by_engine.items():
        if eng == mybir.EngineType.SP:
            continue
        nop = nc.sync.nop()
        for ins in group:
            bass_rust.add_dep_helper(nop.ins, ins, sync=True, reason="absorb eng")


_CACHE = {}
LAST_EXEC_NS = None
LAST_PROFILE = None


def kernel(z, emb, w_lat, b_lat, w_out, b_out, lstm_params):
    z = np.asarray(z, np.float32)
    emb = np.asarray(emb, np.float32)
    w_lat = np.asarray(w_lat, np.float32)
    b_lat = np.asarray(b_lat, np.float32)
    w_out = np.asarray(w_out, np.float32)
    b_out = np.asarray(b_out, np.float32)
    lstm_params = tuple(tuple(np.asarray(a, np.float32) for a in p) for p in lstm_params)

    blob, L = _build_blob(emb, w_lat, b_lat, w_out, b_out, lstm_params)

    import hashlib
    key = hashlib.sha1(blob.tobytes()).hexdigest()
    if key not in _CACHE:
        _CACHE.clear()
        _CACHE[key] = _build_bass(blob, L.ranges)
    nc = _CACHE[key]

    in_maps = []
    for c in range(NCORES):
        zl = z[c * BL:(c + 1) * BL]                       # [256, 512]
        ztp = np.ascontiguousarray(
            zl.reshape(BL, 4, 128).transpose(2, 1, 0).reshape(128, 4 * BL))
        in_maps.append({"zt_in": ztp})

    import os
    trace = bool(os.environ.get("KERNEL_TRACE"))
    res = run_bass_kernel_spmd(nc, in_maps, core_ids=list(range(NCORES)), trace=trace)
    global LAST_EXEC_NS, LAST_PROFILE
    LAST_EXEC_NS = getattr(res, "exec_time_ns", None)
    LAST_PROFILE = getattr(res, "profile_json", None)
    toks = np.concatenate([res.results[c]["out_tokens"] for c in range(NCORES)], axis=0)
    pads = np.concatenate([res.results[c]["out_pads"] for c in range(NCORES)], axis=0)
    return toks.astype(np.int32), pads.astype(np.int32)
